# revision 48
# baseline (speedup 1.0000x reference)
"""GNN message-passing block on 8 Trainium2 NeuronCores.

Two-phase strategy (c-sharded, f1-streamed):
- Phase 1 (tiny NEFF): each core computes f1 = relu(W1^T detF + b1) for its
  own 6250 dets ([32, DC] fp16 out).  The host then performs the neighbor
  gather f1[nIdxs] as pure data movement (index shuffle + strip packing) --
  4x less stream traffic than expanding detF[nIdxs] (32 feats vs 128).
- Phase 2 (main NEFF): per 2048-pair supertile,
    z1 = Wp^T pairF + Wc^T f1[center] (broadcast AP) + Wn^T f1n (streamed)
  then h1 = relu(z1+bp0), z2 = Wp1^T h1, segment-max (strided reduce),
  post-max MLP + output FC + residual + relu.  Phase-3 tiles are interleaved
  into the supertile loop to avoid an end tail.
- Feature-major layout throughout; 4 pair-tiles of 512 pairs pack into one
  [128 x 1024] supertile so DVE/ACT run full-width and the PE uses
  row/col tile_position packing for concurrency.
"""

import sys

sys.path.insert(0, "/opt/trn_rl_repo")

import numpy as np

import concourse.bass as bass
import concourse.tile as tile
from concourse import bacc, mybir
from concourse.bass_utils import run_bass_kernel_spmd

F16 = mybir.dt.float16
F32 = mybir.dt.float32

N_DETS = 50000
KN = 32
N_CORES = 8
DC_REAL = N_DETS // N_CORES          # 6250 real dets per core
DC = 6272                            # padded dets per core (98 * 64)
S = DC // 64                         # 98 supertiles (64 dets / 2048 pairs each)
PAIRS = DC * KN                      # 200704 padded pairs per core
F1C = S * 16                         # 1568 cols of f1packed
PC = S * 32                          # 3136 pooled cols
PCP = 3584                           # pooled cols padded to 7*512
PT3 = PCP // 512                     # 7 phase-3 tiles
AX = mybir.AxisListType.X
RELU = mybir.ActivationFunctionType.Relu

_CACHE = {}


def _build_phase1():
    nc = bacc.Bacc("TRN2", target_bir_lowering=False, debug=False)

    detft16 = nc.dram_tensor("detft16", [128, DC], F16, kind="ExternalInput")
    w1 = nc.dram_tensor("w1", [128, 32], F16, kind="ExternalInput")
    b1x4 = nc.dram_tensor("b1x4", [128, 1], F32, kind="ExternalInput")
    f1out = nc.dram_tensor("f1out", [128, F1C], F16, kind="ExternalOutput")

    with tile.TileContext(nc) as tc:
        with tc.tile_pool(name="pp", bufs=1) as pp, \
             tc.tile_pool(name="psy", bufs=2, space="PSUM") as psy:
            warm = pp.tile([128, 1], F32)
            nc.vector.memset(warm[:], 0.0)
            nc.scalar.activation(warm[:], warm[:], RELU, bias=0.0, scale=1.0)

            w1_t = pp.tile([128, 32], F16)
            nc.sync.dma_start(w1_t[:], w1[:])
            b1_t = pp.tile([128, 1], F32)
            nc.sync.dma_start(b1_t[:], b1x4[:])
            det_t = pp.tile([128, DC], F16)
            for c0 in range(0, DC, 2048):
                cn = min(2048, DC - c0)
                nc.sync.dma_start(det_t[:, c0:c0 + cn], detft16[:, c0:c0 + cn])

            # f1pk[32q+f, 16s+i] = relu(W1^T detF[64s+16q+i] + b1)
            f1pk = pp.tile([128, F1C], F16)
            chunks = [(0, 512), (512, 512), (1024, 512), (1536, 32)]
            for c0, cn in chunks:
                ps1 = psy.tile([128, 512], F32, tag="ps1")
                ns = cn // 16
                s0 = c0 // 16
                dview = det_t[:].rearrange("p (s g) -> p s g", g=64)
                for q in range(4):
                    rhs = dview[:, s0:s0 + ns, 16 * q:16 * q + 16]
                    nc.tensor.matmul(ps1[32 * q:32 * q + 32, :cn], w1_t[:], rhs,
                                     start=True, stop=True, tile_position=(0, 32 * q))
                nc.scalar.activation(f1pk[:, c0:c0 + cn], ps1[:, :cn], RELU,
                                     bias=b1_t[:], scale=1.0)
            nc.sync.dma_start(f1out[:], f1pk[:])

    nc.compile()
    return nc


def _build_phase2():
    nc = bacc.Bacc("TRN2", target_bir_lowering=False, debug=False)

    p16 = nc.dram_tensor("p16", [128, PAIRS // 4], F16, kind="ExternalInput")
    f1n16 = nc.dram_tensor("f1n16", [128, PAIRS // 4], F16, kind="ExternalInput")
    # wcomb = [weights(512) | biases-as-f16(8) | f1pk(1568) | ident(128)]
    WCW = 512 + 8 + F1C + 128
    wcomb = nc.dram_tensor("wcomb", [128, WCW], F16, kind="ExternalInput")
    resid16 = nc.dram_tensor("resid16", [128, 2 * PCP], F16, kind="ExternalInput")
    out_t = nc.dram_tensor("out_t", [128, 2 * PCP], F32, kind="ExternalOutput")

    NG = S // 4 + (1 if S % 4 else 0)          # stream groups of 4 supertiles

    with tile.TileContext(nc) as tc:
        with tc.tile_pool(name="persist", bufs=1) as pp, \
             tc.tile_pool(name="f1s", bufs=2) as f1s_p, \
             tc.tile_pool(name="p16p", bufs=2) as p16_p, \
             tc.tile_pool(name="hbuf", bufs=3) as h_p, \
             tc.tile_pool(name="ph3", bufs=2) as ph3_p, \
             tc.tile_pool(name="psz", bufs=2, space="PSUM") as psz, \
             tc.tile_pool(name="psz2", bufs=2, space="PSUM") as psz2:

            groups = {}

            def load_group(g):
                c0 = 2048 * g
                cw = min(2048, (PAIRS // 4) - c0)
                pt = p16_p.tile([128, 2048], F16, tag="p16", name=f"p16g_{g}")
                nc.sync.dma_start(pt[:, :cw], p16[:, c0:c0 + cw])
                ft = f1s_p.tile([128, 2048], F16, tag="f1n", name=f"f1ng_{g}")
                nc.sync.dma_start(ft[:, :cw], f1n16[:, c0:c0 + cw])
                groups[g] = (pt, ft)

            # warm the ACT function table while the first DMAs are in flight
            warm = pp.tile([128, 1], F32)
            nc.vector.memset(warm[:], 0.0)
            nc.scalar.activation(warm[:], warm[:], RELU, bias=0.0, scale=1.0)

            # --- group 0 arrives in per-supertile chunks so compute can start
            # as soon as supertile 0's slice lands; constants ride in between.
            pt0 = p16_p.tile([128, 2048], F16, tag="p16", name="p16g_0")
            ft0 = f1s_p.tile([128, 2048], F16, tag="f1n", name="f1ng_0")
            nc.sync.dma_start(pt0[:, 0:512], p16[:, 0:512])
            nc.sync.dma_start(ft0[:, 0:512], f1n16[:, 0:512])
            wcomb_t = pp.tile([128, WCW], F16)
            nc.sync.dma_start(wcomb_t[:], wcomb[:])
            for c in (512, 1024, 1536):
                nc.sync.dma_start(pt0[:, c:c + 512], p16[:, c:c + 512])
                nc.sync.dma_start(ft0[:, c:c + 512], f1n16[:, c:c + 512])
            groups[0] = (pt0, ft0)
            load_group(1)

            wp4_t = wcomb_t[:, 0:64]
            wc4_t = wcomb_t[:, 64:128]
            wn4_t = wcomb_t[:, 128:192]
            wp1_t = wcomb_t[:, 192:256]
            wq0_t = wcomb_t[:, 256:320]
            wq1_t = wcomb_t[:, 320:384]
            wo_t = wcomb_t[:, 384:512]
            bp0_t = wcomb_t[:, 512:514].bitcast(F32)
            bp1_t = wcomb_t[:, 514:516].bitcast(F32)
            bq0_t = wcomb_t[:, 516:518].bitcast(F32)
            bq1_t = wcomb_t[:, 518:520].bitcast(F32)
            f1pk = wcomb_t[:, 520:520 + F1C]
            id_t = wcomb_t[:, 520 + F1C:520 + F1C + 128]

            pooled_raw = pp.tile([128, PC], F32)
            pooled = pp.tile([128, PC], F16)
            p3state = {}

            # --- phase-3 tile t as 4 stages, each split into an "mm" half
            # (emitted at the TOP of an iteration: the borrowed z2-pool PSUM
            # slot's WAR cleared two MAXes ago, so the PE never blocks) and a
            # "drain" half (emitted AFTER h1 so the critical h1 ACT is never
            # delayed).  Residual add rides the PE as M=64-split identity
            # matmuls; the final relu rides the ACT drain.  No DVE work.
            def p3_s0_mm(t):
                res_t = ph3_p.tile([128, 1024], F16, tag="res", name=f"res_{t}")
                nc.sync.dma_start(res_t[:], resid16[:, 1024 * t:1024 * (t + 1)])
                p3state[("res", t)] = res_t

            def p3_s0_dr(t):
                c = 512 * t
                cn = min(512, PC - c)
                nc.scalar.activation(pooled[:, c:c + cn], pooled_raw[:, c:c + cn],
                                     RELU, bias=bp1_t, scale=1.0)

            def pad_parity(nm):
                # keep the z2 rotation parity: pair each p3 transient alloc
                # with a dummy alloc (tiny memset preserves the WAW chain)
                dum = psz2.tile([128, 1024], F32, tag="z2", name=nm)
                nc.vector.memset(dum[0:1, 0:1], 0.0)

            def p3_s1_mm(t):
                c = 512 * t
                cn = min(512, PC - c)
                pq = psz2.tile([128, 1024], F32, tag="z2", name=f"pq1_{t}")
                pad_parity(f"dum1_{t}")
                nc.tensor.matmul(pq[0:64, :cn], wq0_t[0:64], pooled[0:64, c:c + cn],
                                 start=True, stop=True, tile_position=(0, 0))
                nc.tensor.matmul(pq[64:128, :cn], wq0_t[64:128], pooled[64:128, c:c + cn],
                                 start=True, stop=True, tile_position=(64, 64))
                p3state[("pq1", t)] = pq

            def p3_s1_dr(t):
                cn = min(512, PC - 512 * t)
                pq = p3state.pop(("pq1", t))
                p1 = ph3_p.tile([128, 512], F16, tag="p1", name=f"p1_{t}")
                nc.scalar.activation(p1[:, :cn], pq[:, :cn], RELU, bias=bq0_t, scale=1.0)
                p3state[("p1", t)] = p1

            def p3_s2_mm(t):
                cn = min(512, PC - 512 * t)
                p1 = p3state.pop(("p1", t))
                pq = psz2.tile([128, 1024], F32, tag="z2", name=f"pq2_{t}")
                pad_parity(f"dum2_{t}")
                nc.tensor.matmul(pq[0:64, :cn], wq1_t[0:64], p1[0:64, :cn],
                                 start=True, stop=True, tile_position=(0, 0))
                nc.tensor.matmul(pq[64:128, :cn], wq1_t[64:128], p1[64:128, :cn],
                                 start=True, stop=True, tile_position=(64, 64))
                p3state[("pq2", t)] = pq

            def p3_s2_dr(t):
                cn = min(512, PC - 512 * t)
                pq = p3state.pop(("pq2", t))
                p2 = ph3_p.tile([128, 512], F16, tag="p2", name=f"p2_{t}")
                nc.scalar.activation(p2[:, :cn], pq[:, :cn], RELU, bias=bq1_t, scale=1.0)
                p3state[("p2", t)] = p2

            def p3_s3_mm(t):
                cn = min(512, PC - 512 * t)
                p2 = p3state.pop(("p2", t))
                res_t = p3state.pop(("res", t))
                rf = psz2.tile([128, 1024], F32, tag="z2", name=f"rf_{t}")
                pad_parity(f"dum3_{t}")
                for h in range(2):
                    for m in range(2):
                        rc = rf[64 * m:64 * m + 64, 512 * h:512 * h + cn]
                        nc.tensor.matmul(
                            rc, id_t[:, 64 * m:64 * m + 64],
                            res_t[:, 512 * h:512 * h + cn],
                            start=True, stop=False,
                            tile_position=(0, 64 * m),
                            skip_group_check=True)
                        nc.tensor.matmul(
                            rc, wo_t[64 * h:64 * h + 64, 64 * m:64 * m + 64],
                            p2[64 * h:64 * h + 64, :cn],
                            start=False, stop=True,
                            tile_position=(64 * h, 64 * m),
                            skip_group_check=True)
                p3state[("rf", t)] = rf

            def p3_s3_dr(t):
                cn = min(512, PC - 512 * t)
                rf = p3state.pop(("rf", t))
                o_sb = ph3_p.tile([128, 1024], F32, tag="osb", name=f"osb_{t}")
                for h in range(2):
                    nc.scalar.activation(o_sb[:, 512 * h:512 * h + cn],
                                         rf[:, 512 * h:512 * h + cn],
                                         RELU, bias=0.0, scale=1.0)
                    nc.sync.dma_start(
                        out_t[:, 1024 * t + 512 * h:1024 * t + 512 * h + cn],
                        o_sb[:, 512 * h:512 * h + cn])

            P3_STAGES = ((p3_s0_mm, p3_s0_dr), (p3_s1_mm, p3_s1_dr),
                         (p3_s2_mm, p3_s2_dr), (p3_s3_mm, p3_s3_dr))
            pending = []           # entries: [due, stage_idx, t, mm_done]
            drained = {}           # t -> last stage whose drain was emitted

            def flush_mm(it):
                for e in pending:
                    due, i, t, mm_done = e
                    if due <= it and not mm_done and \
                            (i == 0 or drained.get(t) == i - 1):
                        P3_STAGES[i][0](t)
                        e[3] = True

            def flush_dr(it):
                for e in list(pending):
                    due, i, t, mm_done = e
                    if mm_done:
                        P3_STAGES[i][1](t)
                        drained[t] = i
                        pending.remove(e)

            # --- main loop: supertiles of 2048 pairs (4 tiles x 512)
            prev = None
            n_emitted = 0
            for s in range(S):
                # p3 stage matmuls first (PSUM WAR already clear -> no PE
                # block); their ACT drains are emitted after h1 below.
                flush_mm(n_emitted)
                if s % 4 == 0:
                    g = s // 4
                    if g + 1 < NG and (g + 1) not in groups:
                        load_group(g + 1)
                    p16_t, f1n_t = groups[g]
                    groups.pop(g - 1, None)

                # z1 accumulation: 3 matmuls per quarter, round-robin across quarters
                z1 = psz.tile([128, 1024], F32, tag="z1", name=f"z1_{s}")
                pcol = 512 * (s % 4)
                for q in range(4):
                    tp = (32 * q, 64 * (q % 2))
                    o = z1[64 * (q % 2):64 * (q % 2) + 64, 512 * (q // 2):512 * (q // 2) + 512]
                    nc.tensor.matmul(o, wp4_t[32 * q:32 * q + 32, :],
                                     p16_t[32 * q:32 * q + 32, pcol:pcol + 512],
                                     start=True, stop=False, tile_position=tp,
                                     skip_group_check=True)
                for q in range(4):
                    tp = (32 * q, 64 * (q % 2))
                    o = z1[64 * (q % 2):64 * (q % 2) + 64, 512 * (q // 2):512 * (q // 2) + 512]
                    rhs = f1pk[32 * q:32 * q + 32, 16 * s:16 * s + 16].rearrange(
                        "p (d one) -> p d one", one=1).to_broadcast([32, 16, 32])
                    nc.tensor.matmul(o, wc4_t[32 * q:32 * q + 32, :], rhs,
                                     start=False, stop=False, tile_position=tp,
                                     skip_group_check=True)
                for q in range(4):
                    tp = (32 * q, 64 * (q % 2))
                    o = z1[64 * (q % 2):64 * (q % 2) + 64, 512 * (q // 2):512 * (q // 2) + 512]
                    nc.tensor.matmul(o, wn4_t[32 * q:32 * q + 32, :],
                                     f1n_t[32 * q:32 * q + 32, pcol:pcol + 512],
                                     start=False, stop=True, tile_position=tp,
                                     skip_group_check=True)

                # h1 = relu(z1 + bp0) on ACT; z2+segmax for the PREVIOUS supertile
                # are emitted after this supertile's z1 matmuls (software
                # pipelining) so the PE never head-blocks waiting for ACT.
                h1 = h_p.tile([128, 1024], F16, tag="h1", name=f"h1_{s}")
                nc.scalar.activation(h1[:], z1[:], RELU, bias=bp0_t, scale=1.0)
                flush_dr(n_emitted)

                def emit_l2(sp, h1p):
                    # layer 2 + segment max; max(relu(z+b)) == relu(max(z)+b):
                    # relu+bias deferred to the pooled array.
                    z2 = psz2.tile([128, 1024], F32, tag="z2", name=f"z2_{sp}")
                    for q in range(4):
                        hp = 64 * (q % 2)
                        cp = 512 * (q // 2)
                        nc.tensor.matmul(z2[hp:hp + 64, cp:cp + 512],
                                         wp1_t[hp:hp + 64, :],
                                         h1p[hp:hp + 64, cp:cp + 512],
                                         start=True, stop=True, tile_position=(hp, hp))
                    src = z2[:].rearrange("p (d k) -> p d k", k=32)
                    dst = pooled_raw[:, 32 * sp:32 * sp + 32].rearrange(
                        "p (d one) -> p d one", one=1)
                    nc.vector.tensor_reduce(dst, src, op=mybir.AluOpType.max, axis=AX)

                if prev is not None:
                    emit_l2(*prev)
                    n_emitted += 1
                    sp = prev[0]
                    if (sp + 1) % 16 == 0 and (sp + 1) // 16 <= 6:
                        t = (sp + 1) // 16 - 1
                        for i in range(4):
                            pending.append([n_emitted + i, i, t, False])
                prev = (s, h1)
            emit_l2(*prev)
            n_emitted += 1
            for i in range(4):
                pending.append([n_emitted + i, i, 6, False])
            it = n_emitted
            while pending:
                flush_mm(it)
                flush_dr(it)
                it += 1

    nc.compile()
    return nc


def _dets_of_core(k):
    return np.arange(DC_REAL * k, DC_REAL * (k + 1))


def _host_prep_phase1(detFeatures, W1, b1):
    f16 = np.float16
    detF = np.asarray(detFeatures, np.float32)
    W1_16 = np.ascontiguousarray(W1, np.float32).astype(f16)           # [128, 32]
    b1x4 = np.tile(np.asarray(b1, np.float32), 4)[:, None]             # [128, 1]
    in_maps = []
    for k in range(N_CORES):
        dpad = np.zeros((DC, 128), np.float32)
        dpad[:DC_REAL] = detF[_dets_of_core(k)]
        detft16 = np.ascontiguousarray(dpad.T.astype(f16))             # [128, DC]
        in_maps.append({"detft16": detft16, "w1": W1_16, "b1x4": b1x4})
    return in_maps


def _unpack_f1(f1pk):
    """[128, F1C] packed -> [DC, 32] det-major (row 32q+f, col 16s+i)."""
    return f1pk.reshape(4, 32, S, 16).transpose(2, 0, 3, 1).reshape(DC, 32)


def _host_prep_phase2(f1_full, f1pks, detFeatures, nIdxs, pairFeatures,
                      Wp0, bp0, Wp1, bp1, Wq0, bq0, Wq1, bq1, Wo, bo):
    """f1_full: [N_DETS, 32] fp16; f1pks: per-core packed [128, F1C] fp16.
    Returns (in_maps, out_col)."""
    f16 = np.float16
    detF = np.asarray(detFeatures, np.float32)
    pairF = np.asarray(pairFeatures, np.float32)
    nI = np.asarray(nIdxs, np.int64)

    wcat = np.concatenate([
        np.tile(Wp0[0:32].astype(f16), (4, 1)),                        # wp4
        np.tile(Wp0[32:64].astype(f16), (4, 1)),                       # wc4
        np.tile(Wp0[64:96].astype(f16), (4, 1)),                       # wn4
        np.tile(Wp1.astype(f16), (2, 1)),                              # wp1
        np.tile(Wq0.astype(f16), (2, 1)),                              # wq0
        np.tile(Wq1.astype(f16), (2, 1)),                              # wq1
        np.tile(Wo.astype(f16), (2, 1)),                               # wo
    ], axis=1)                                                         # [128, 512]
    bcat = np.stack([
        np.tile(np.asarray(bp0, np.float32), 2),
        np.tile(np.asarray(bp1, np.float32), 2),
        np.tile(np.asarray(bq0, np.float32), 2),
        np.tile(np.asarray(bq1, np.float32), 2),
    ], axis=1)                                                         # [128, 4] f32
    bo32 = np.asarray(bo, np.float32)

    # det-order scramble for pooled/output columns:
    # local det d: s = d//64, q = (d%64)//16, i = d%16
    d = np.arange(DC)
    s_, q_, i_ = d // 64, (d % 64) // 16, d % 16
    pooled_col = 32 * s_ + 16 * (q_ // 2) + i_
    half = q_ % 2
    t3 = pooled_col // 512
    out_col = 1024 * t3 + 512 * half + (pooled_col % 512)              # [DC]

    in_maps = []
    for k in range(N_CORES):
        dets = _dets_of_core(k)
        dloc = detF[dets]                                              # [6250, 128]

        # resid16[:, out_col[d]] = detF[d] + bo  (scrambled; pads zero)
        resid = np.zeros((2 * PCP, 128), f16)
        resid[out_col[:DC_REAL]] = (dloc + bo32).astype(f16)
        resid16 = np.ascontiguousarray(resid.T)                        # [128, 2*PCP]

        # pairs of this core, padded
        pf = np.zeros((PAIRS, 32), np.float32)
        pf[:DC_REAL * KN] = pairF[DC_REAL * KN * k: DC_REAL * KN * (k + 1)]
        # strip packing: [S, 4, 512, 32] -> [4, 32, S, 512] -> [128, S*512]
        p16 = np.ascontiguousarray(
            pf.reshape(S, 4, 512, 32).transpose(1, 3, 0, 2).reshape(128, S * 512)
        ).astype(f16)

        # neighbor f1 stream, same strip packing
        ni = np.zeros(PAIRS, np.int64)
        ni[:DC_REAL * KN] = nI[DC_REAL * KN * k: DC_REAL * KN * (k + 1)]
        f1n = f1_full[ni]                                              # [PAIRS, 32] f16
        f1n16 = np.ascontiguousarray(
            f1n.reshape(S, 4, 512, 32).transpose(1, 3, 0, 2).reshape(128, S * 512))

        # wcomb = [weights | biases bitcast to f16 | f1pk (phase-1 packed) | I]
        wcomb = np.concatenate(
            [wcat, bcat.astype(np.float32).view(f16), f1pks[k],
             np.eye(128, dtype=f16)], axis=1)

        in_maps.append({
            "p16": p16, "f1n16": f1n16, "wcomb": wcomb, "resid16": resid16,
        })
    return in_maps, out_col


def _run(inputs, trace=False):
    if "nc1" not in _CACHE:
        _CACHE["nc1"] = _build_phase1()
    if "nc2" not in _CACHE:
        _CACHE["nc2"] = _build_phase2()
    nc1, nc2 = _CACHE["nc1"], _CACHE["nc2"]

    in1 = _host_prep_phase1(inputs["detFeatures"], inputs["W1"], inputs["b1"])
    res1 = run_bass_kernel_spmd(nc1, in1, core_ids=list(range(N_CORES)),
                                trace=trace)
    f1pks = [res1.results[k]["f1out"] for k in range(N_CORES)]
    f1_full = np.concatenate(
        [_unpack_f1(f1pks[k])[:DC_REAL] for k in range(N_CORES)], axis=0)

    in2, out_col = _host_prep_phase2(
        f1_full, f1pks, inputs["detFeatures"], inputs["nIdxs"], inputs["pairFeatures"],
        inputs["Wp0"], inputs["bp0"], inputs["Wp1"], inputs["bp1"],
        inputs["Wq0"], inputs["bq0"], inputs["Wq1"], inputs["bq1"],
        inputs["Wo"], inputs["bo"])
    res2 = run_bass_kernel_spmd(nc2, in2, core_ids=list(range(N_CORES)),
                                trace=trace)
    outs = []
    for k in range(N_CORES):
        ot = res2.results[k]["out_t"]                                  # [128, 2*PCP]
        outs.append(ot[:, out_col[:DC_REAL]].T)                        # [6250, 128]
    full = np.concatenate(outs, axis=0).astype(np.float32)
    return full, (res1, res2)


def kernel(**inputs):
    inputs = {k: np.asarray(v) for k, v in inputs.items()}
    full, _ = _run(inputs, trace=False)
    return full
